# Initial kernel scaffold
#
"""GSOccLocalBridge Trainium2 kernel.

Computation (see reference):
  gs_to_occ = scatter_mean(voxelize(centers), feats @ W_g2o + b_g2o)   [BT,Co,NZ,NY,NX]
  occ_to_gs = trilinear(occ, centers) @ W_o2g + b_o2g                  [BT,N,Cg]

Both Linear layers commute with the (linear) scatter/gather, so on device we:
  - scatter-mean the raw 48-dim feats (+1 occupancy col) into voxel windows via
    one-hot matmuls on PE, then project 49->256 per window and write straight
    into the channel-major output layout (no transposes, no indirect DMA).
  - project occ 256->48 first (channel-major layout is already matmul-ready),
    PE-transpose the small projected volume into row-major [vox,48(+pad)] in
    DRAM, then dma_gather 512B x-pair rows per trilinear corner and do the
    weighted corner sum on DVE.

Sharding: 8 cores = 2 (b,t) x 4 y-quarters. One SPMD program; all per-core
differences are data (host-built index/weight tables from the point geometry).
"""

import sys

if "/opt/trn_rl_repo" not in sys.path:
    sys.path.insert(0, "/opt/trn_rl_repo")

import numpy as np

# ---------------- problem constants (hardcoded from spec) ----------------
GRID_MIN = np.array([-40.0, -40.0, -2.0], dtype=np.float32)
VOXEL = np.float32(0.8)
NZ, NY, NX = 10, 100, 100
NVOX = NZ * NY * NX
B, T, V, H, W_, Cg, Co = 1, 2, 6, 32, 88, 48, 256
BT, NPTS = B * T, V * H * W_  # 2, 16896

KQ = 4             # y-quarter cores per (b,t)
YQ = 25            # y rows owned per quarter (scatter)
YROWS = YQ + 1     # occP slice rows per z (gather needs y0+1)
ROWLEN = YROWS * NX          # 2600 contiguous elems per (ch,z) in occ_in
SLICE = NZ * ROWLEN          # 26000 rows in occP
SCROW = YQ * NX              # 2500 scatter-owned elems per z
PCH = [(c, min(512, ROWLEN - c)) for c in range(0, ROWLEN, 512)]   # proj chunks
SWIN = [(c, min(512, SCROW - c)) for c in range(0, SCROW, 512)]    # scatter windows
OCCP_COLS = 64     # 48 data + 16 pad so a (x,x0+1) pair row is 512B
F32 = np.float32


# ---------------- host-side geometry prep ----------------

def _voxel_ijk(pts):
    # identical formula to reference._scatter_mean, fp32 throughout
    q = (pts - GRID_MIN[None, :]) / VOXEL
    ijk = np.floor(q).astype(np.int32)
    return ijk


def _trilinear_geom(pts):
    # identical formula to reference._trilinear_sample, fp32 throughout
    dims = np.array([NX * 0.8, NY * 0.8, NZ * 0.8], dtype=np.float32)
    sizes = np.array([NX, NY, NZ], dtype=np.float32)
    g = (pts - GRID_MIN[None, :]) / np.maximum(dims, np.float32(1e-6)) * np.float32(2.0) - np.float32(1.0)
    pix = ((g + np.float32(1.0)) * sizes[None, :] - np.float32(1.0)) * np.float32(0.5)
    p0 = np.floor(pix)
    frac = (pix - p0).astype(np.float32)
    return p0.astype(np.int32), frac


def host_prep(centers, gs_features):
    """Build per-core input tensors + the (input-derived) static program meta."""
    pts = centers.reshape(BT, NPTS, 3)
    feats = gs_features.reshape(BT, NPTS, Cg)

    # ---- scatter geometry ----
    sc_buckets = {}   # (bt,k) -> dict (z,w) -> list of point idx
    sc_scale = np.zeros((BT, NPTS), dtype=F32)
    sc_zwu = np.zeros((BT, NPTS, 3), dtype=np.int32)   # z, w, urel per point
    sc_core = -np.ones((BT, NPTS), dtype=np.int32)
    for bt in range(BT):
        ijk = _voxel_ijk(pts[bt])
        ix, iy, iz = ijk[:, 0], ijk[:, 1], ijk[:, 2]
        valid = (ix >= 0) & (ix < NX) & (iy >= 0) & (iy < NY) & (iz >= 0) & (iz < NZ)
        lin = (np.clip(iz, 0, NZ - 1) * (NY * NX)
               + np.clip(iy, 0, NY - 1) * NX
               + np.clip(ix, 0, NX - 1))
        cnt = np.bincount(lin[valid], minlength=NVOX).astype(F32)
        sc_scale[bt, valid] = np.float32(1.0) / cnt[lin[valid]]
        k = iy // YQ
        uloc = (iy - k * YQ) * NX + ix
        w = np.minimum(uloc // 512, len(SWIN) - 1)
        sc_zwu[bt, :, 0] = iz
        sc_zwu[bt, :, 1] = w
        sc_zwu[bt, :, 2] = uloc - w * 512
        sc_core[bt, valid] = k[valid]
        for kk in range(KQ):
            d = {}
            sel = np.nonzero(valid & (k == kk))[0]
            for n in sel:
                d.setdefault((iz[n], w[n]), []).append(n)
            sc_buckets[(bt, kk)] = d

    # uniform rounds grid
    rgrid = np.zeros((NZ, len(SWIN)), dtype=np.int64)
    for z in range(NZ):
        for w in range(len(SWIN)):
            m = max(len(sc_buckets[(bt, kk)].get((z, w), []))
                    for bt in range(BT) for kk in range(KQ))
            rgrid[z, w] = max(1, -(-m // 128))
    # chunk meta in emission order: (z, w, w0, W, r, R)
    sc_chunks = []
    for z in range(NZ):
        for w, (w0, Ww) in enumerate(SWIN):
            R = int(rgrid[z, w])
            for r in range(R):
                sc_chunks.append((z, w, w0, Ww, r, R))
    NCHUNK_SC = len(sc_chunks)

    # ---- gather geometry ----
    g_bucket_pts = {}
    gp0 = np.zeros((BT, NPTS, 3), dtype=np.int32)
    gfr = np.zeros((BT, NPTS, 3), dtype=F32)
    for bt in range(BT):
        p0, frac = _trilinear_geom(pts[bt])
        gp0[bt], gfr[bt] = p0, frac
        k = np.clip(p0[:, 1] // YQ, 0, KQ - 1)
        for kk in range(KQ):
            g_bucket_pts[(bt, kk)] = np.nonzero(k == kk)[0]
    NT_G = max(-(-len(g_bucket_pts[(bt, kk)]) // 128)
               for bt in range(BT) for kk in range(KQ))
    NB_G = -(-NT_G // 4)

    meta = dict(sc_chunks=sc_chunks, NCHUNK_SC=NCHUNK_SC, NB_G=NB_G)

    # ---- per-core tensors ----
    in_maps = []
    perms = []
    for core in range(8):
        bt, kk = core // KQ, core % KQ

        # fsu: [NCHUNK_SC, 128, 52] = feats(48) | one(1) | scale(1) | urel(1) | pad(1)
        fsu = np.zeros((NCHUNK_SC, 128, 52), dtype=F32)
        bk = sc_buckets[(bt, kk)]
        for c, (z, w, w0, Ww, r, R) in enumerate(sc_chunks):
            lst = bk.get((z, w), [])
            part = lst[r * 128:(r + 1) * 128]
            if part:
                idxs = np.asarray(part, dtype=np.int64)
                npart = len(idxs)
                fsu[c, :npart, :Cg] = feats[bt, idxs]
                fsu[c, :npart, Cg] = 1.0
                fsu[c, :npart, Cg + 1] = sc_scale[bt, idxs]
                fsu[c, :npart, Cg + 2] = sc_zwu[bt, idxs, 2].astype(F32)

        # gather tables
        pidx = g_bucket_pts[(bt, kk)]
        perms.append(pidx)
        npt = len(pidx)
        gidx = np.zeros((NB_G, 128, 128), dtype=np.int16)
        gw = np.zeros((NB_G, 128, 32), dtype=F32)
        p0 = gp0[bt][pidx]
        fr = gfr[bt][pidx]
        x0, y0, z0 = p0[:, 0], p0[:, 1], p0[:, 2]
        fx, fy, fz = fr[:, 0], fr[:, 1], fr[:, 2]
        xp = np.clip(x0, 0, NX - 2)
        for cp, (dz, dy) in enumerate(((0, 0), (0, 1), (1, 0), (1, 1))):
            zc = z0 + dz
            yc = y0 + dy
            okzy = ((zc >= 0) & (zc < NZ) & (yc >= 0) & (yc < NY)).astype(F32)
            wz = np.where(dz == 1, fz, np.float32(1.0) - fz).astype(F32)
            wy = np.where(dy == 1, fy, np.float32(1.0) - fy).astype(F32)
            idx = (np.clip(zc, 0, NZ - 1) * ROWLEN
                   + np.clip(yc - kk * YQ, 0, YROWS - 1) * NX + xp).astype(np.int64)
            for s in range(2):
                xs = xp + s
                wx = np.where(xs == x0, np.float32(1.0) - fx,
                              np.where(xs == x0 + 1, fx, np.float32(0.0))).astype(F32)
                wfull = (wx * wy * wz * okzy).astype(F32)
                for n in range(npt):
                    tg = n // 128          # global tile
                    b = tg // 4            # batch
                    t = tg % 4             # tile in batch
                    p = n % 128
                    i = (t * 4 + cp) * 128 + p     # slot in batch
                    gw[b, p, t * 8 + cp * 2 + s] = wfull[n]
                    if s == 0:
                        gidx[b, (i % 16) + 16 * 0, i // 16] = idx[n]
            # replicate wrapped idx rows across the 8 gpsimd core groups
        for rr in range(1, 8):
            gidx[:, 16 * rr:16 * rr + 16, :] = gidx[:, 0:16, :]

        # occ_in filled by caller (needs occ_volume)
        in_maps.append(dict(fsu=fsu, gidx=gidx, gw=gw))

    return in_maps, perms, meta


# ---------------- device kernel builder ----------------

def build_kernel(meta):
    import concourse.bass as bass
    import concourse.mybir as mybir
    import concourse.tile as tile
    from concourse import bacc
    from concourse.masks import make_identity

    f32, f32r = mybir.dt.float32, mybir.dt.float32r
    i16, i32 = mybir.dt.int16, mybir.dt.int32
    AOP = mybir.AluOpType
    sc_chunks, NB_G = meta["sc_chunks"], meta["NB_G"]
    NCHUNK_SC = meta["NCHUNK_SC"]

    nc = bacc.Bacc(None, target_bir_lowering=False)

    occ_in = nc.dram_tensor("occ_in", [Co, NZ, YROWS, NX], f32, kind="ExternalInput")
    fsu_t = nc.dram_tensor("fsu", [NCHUNK_SC, 128, 52], f32, kind="ExternalInput")
    gidx_t = nc.dram_tensor("gidx", [NB_G, 128, 128], i16, kind="ExternalInput")
    gw_t = nc.dram_tensor("gw", [NB_G, 128, 32], f32, kind="ExternalInput")
    wp_t = nc.dram_tensor("wp", [Cg + 1, Co], f32, kind="ExternalInput")
    wo_t = nc.dram_tensor("wo", [2, 128, Cg], f32, kind="ExternalInput")
    brep_t = nc.dram_tensor("brep", [128, 4, Cg], f32, kind="ExternalInput")

    out_sc = nc.dram_tensor("out_sc", [2, 128, NZ, SCROW], f32, kind="ExternalOutput")
    out_g = nc.dram_tensor("out_g", [NB_G * 512, Cg], f32, kind="ExternalOutput")
    occp = nc.dram_tensor("occp", [SLICE + 1, OCCP_COLS], f32, kind="Internal")

    with tile.TileContext(nc) as tc:
        with (
            tc.tile_pool(name="const", bufs=1) as cpool,
            tc.tile_pool(name="proj", bufs=3) as ppool,
            tc.tile_pool(name="scat", bufs=3) as spool,
            tc.tile_pool(name="gath", bufs=2) as gpool,
            tc.tile_pool(name="ps_proj", bufs=2, space="PSUM") as ps_proj,
            tc.tile_pool(name="ps_scat", bufs=2, space="PSUM") as ps_scat,
        ):
            # ---- constants ----
            ident = cpool.tile([128, 128], f32)
            make_identity(nc, ident[:])
            iota_i = cpool.tile([128, 512], i32)
            nc.gpsimd.iota(iota_i[:], pattern=[[1, 512]], base=0, channel_multiplier=0)
            iota_f = cpool.tile([128, 512], f32)
            nc.vector.tensor_copy(iota_f[:], iota_i[:])
            wp = cpool.tile([Cg + 1, Co], f32)
            nc.sync.dma_start(wp[:], wp_t[:])
            wo = cpool.tile([128, 2, Cg], f32)
            nc.sync.dma_start(wo[:], wo_t[:].rearrange("h p c -> p h c"))
            brep = cpool.tile([128, 4, Cg], f32)
            nc.sync.dma_start(brep[:], brep_t[:])

            occ_view = occ_in[:].rearrange("(h c) z y x -> c h z (y x)", h=2)

            # ---- occ -> occP projection (256 -> 48, then transpose to rows) ----
            for z in range(NZ):
                for c0, Wc in PCH:
                    oc = ppool.tile([128, 2, Wc], f32, tag="oc")
                    nc.sync.dma_start(oc[:], occ_view[:, :, z, c0:c0 + Wc])
                    pj = ps_proj.tile([Cg, Wc], f32, tag="pj")
                    for h in range(2):
                        nc.tensor.matmul(
                            pj[:],
                            lhsT=wo[:, h, :].bitcast(f32r),
                            rhs=oc[:, h, :].bitcast(f32r),
                            start=(h == 0), stop=(h == 1),
                        )
                    s1 = ppool.tile([Cg, Wc], f32, tag="s1")
                    nc.scalar.copy(s1[:], pj[:])
                    nb = -(-Wc // 128)
                    pT = ps_proj.tile([128, nb, Cg], f32, tag="pT")
                    for t in range(nb):
                        wseg = min(128, Wc - t * 128)
                        nc.tensor.transpose(
                            pT[:wseg, t, :],
                            s1[:, t * 128:t * 128 + wseg],
                            ident[:Cg, :Cg],
                        )
                    sT = ppool.tile([128, nb, Cg], f32, tag="sT")
                    nc.vector.tensor_copy(sT[:], pT[:])
                    r0 = z * ROWLEN + c0
                    dst = bass.AP(occp[:].tensor, r0 * OCCP_COLS,
                                  [[OCCP_COLS, 128], [128 * OCCP_COLS, nb], [1, Cg]])
                    nc.sync.dma_start(dst, sT[:])

            # ---- scatter: one-hot matmul accumulate + project per window ----
            widx = 0
            for ci, (z, w, w0, Ww, r, R) in enumerate(sc_chunks):
                ft = spool.tile([128, 52], f32, tag="ft")
                nc.sync.dma_start(ft[:], fsu_t[ci])
                fs = spool.tile([128, Cg + 1], f32, tag="fs")
                nc.vector.tensor_scalar(
                    out=fs[:], in0=ft[:, :Cg + 1],
                    scalar1=ft[:, Cg + 1:Cg + 2], scalar2=None, op0=AOP.mult)
                oh = spool.tile([128, Ww], f32, tag="oh")
                nc.vector.tensor_scalar(
                    out=oh[:], in0=iota_f[:, :Ww],
                    scalar1=ft[:, Cg + 2:Cg + 3], scalar2=None, op0=AOP.is_equal)
                if r == 0:
                    psw = ps_scat.tile([Cg + 1, Ww], f32, tag="psw")
                    meta.setdefault("_psw", {})[ci - r] = psw
                psw = meta["_psw"][ci - r]
                nc.tensor.matmul(
                    psw[:], lhsT=fs[:].bitcast(f32r), rhs=oh[:].bitcast(f32r),
                    start=(r == 0), stop=(r == R - 1))
                if r == R - 1:
                    ws = spool.tile([Cg + 1, Ww], f32, tag="ws")
                    nc.scalar.copy(ws[:], psw[:])
                    for h in range(2):
                        p2 = ps_scat.tile([128, Ww], f32, tag="p2")
                        nc.tensor.matmul(
                            p2[:], lhsT=wp[:, h * 128:(h + 1) * 128].bitcast(f32r),
                            rhs=ws[:].bitcast(f32r), start=True, stop=True)
                        nc.sync.dma_start(out_sc[h, :, z, w0:w0 + Ww], p2[:])
                    widx += 1

            # ---- gather: dma_gather corner pairs + DVE weighted sum ----
            gsrc = bass.AP(occp[:].tensor, 0, [[OCCP_COLS, SLICE], [1, 128]])
            for b in range(NB_G):
                it = gpool.tile([128, 128], i16, tag="it")
                nc.sync.dma_start(it[:], gidx_t[b])
                wt = gpool.tile([128, 32], f32, tag="wt")
                nc.sync.dma_start(wt[:], gw_t[b])
                g = gpool.tile([128, 16, 128], f32, tag="g")
                nc.gpsimd.dma_gather(
                    g[:], gsrc, it[:], num_idxs=2048, num_idxs_reg=2048,
                    elem_size=128, elem_step=OCCP_COLS, queue_num=b % 4)
                gv = g[:].rearrange("p (t cp) e -> p t cp e", cp=4)
                wv = wt[:].rearrange("p (t q) -> p t q", q=8)
                acc = gpool.tile([128, 4, Cg], f32, tag="acc")
                tmp = gpool.tile([128, 4, Cg], f32, tag="tmp")
                first = True
                for cp in range(4):
                    for s in range(2):
                        gsl = gv[:, :, cp, s * 64:s * 64 + Cg]
                        wsl = wv[:, :, cp * 2 + s:cp * 2 + s + 1].to_broadcast([128, 4, Cg])
                        if first:
                            nc.vector.tensor_tensor(out=acc[:], in0=gsl, in1=wsl, op=AOP.mult)
                            first = False
                        else:
                            nc.vector.tensor_tensor(out=tmp[:], in0=gsl, in1=wsl, op=AOP.mult)
                            nc.vector.tensor_tensor(out=acc[:], in0=acc[:], in1=tmp[:], op=AOP.add)
                nc.vector.tensor_tensor(out=acc[:], in0=acc[:], in1=brep[:], op=AOP.add)
                dstg = out_g[:].rearrange("(b t p) c -> b p t c", t=4, p=128)[b]
                nc.sync.dma_start(dstg, acc[:])

    meta.pop("_psw", None)
    nc.compile()
    return nc


# ---------------- top-level entry ----------------

def kernel(**inputs):
    centers = np.asarray(inputs["centers"], dtype=np.float32)
    gs_features = np.asarray(inputs["gs_features"], dtype=np.float32)
    occ_volume = np.asarray(inputs["occ_volume"], dtype=np.float32)
    W_g2o = np.asarray(inputs["W_g2o"], dtype=np.float32)
    b_g2o = np.asarray(inputs["b_g2o"], dtype=np.float32)
    W_o2g = np.asarray(inputs["W_o2g"], dtype=np.float32)
    b_o2g = np.asarray(inputs["b_o2g"], dtype=np.float32)

    in_maps, perms, meta = host_prep(centers, gs_features)

    wp = np.concatenate([W_g2o, b_g2o[None, :]], axis=0).astype(F32)       # [49,256]
    wo = W_o2g.reshape(2, 128, Cg).astype(F32)                             # [2,128,48]
    brep = np.broadcast_to(b_o2g[None, None, :], (128, 4, Cg)).astype(F32).copy()

    occ = occ_volume.reshape(BT, Co, NZ, NY, NX)
    for core in range(8):
        bt, kk = core // KQ, core % KQ
        sl = np.zeros((Co, NZ, YROWS, NX), dtype=F32)
        ylo = kk * YQ
        yhi = min(ylo + YROWS, NY)
        sl[:, :, :yhi - ylo, :] = occ[bt, :, :, ylo:yhi, :]
        in_maps[core].update(occ_in=sl, wp=wp, wo=wo, brep=brep)

    nc = build_kernel(meta)

    from concourse import bass_utils
    res = bass_utils.run_bass_kernel_spmd(nc, in_maps, core_ids=list(range(8)))

    gs_to_occ = np.zeros((B, T, Co, NZ, NY, NX), dtype=F32)
    occ_to_gs_flat = np.zeros((BT, NPTS, Cg), dtype=F32)
    for core in range(8):
        bt, kk = core // KQ, core % KQ
        osc = res.results[core]["out_sc"].reshape(Co, NZ, YQ, NX)
        gs_to_occ[0, bt, :, :, kk * YQ:(kk + 1) * YQ, :] = osc
        og = res.results[core]["out_g"]
        pidx = perms[core]
        occ_to_gs_flat[bt, pidx] = og[:len(pidx)]
    occ_to_gs = occ_to_gs_flat.reshape(B, T, V, H, W_, Cg)
    return gs_to_occ, occ_to_gs


# revision 6
# speedup vs baseline: 1.0619x; 1.0619x over previous
"""GSOccLocalBridge Trainium2 kernel.

Computation (see reference):
  gs_to_occ = scatter_mean(voxelize(centers), feats @ W_g2o + b_g2o)   [BT,Co,NZ,NY,NX]
  occ_to_gs = trilinear(occ, centers) @ W_o2g + b_o2g                  [BT,N,Cg]

Both Linear layers commute with the (linear) scatter/gather, so on device we:
  - scatter-mean the raw 48-dim feats (+1 occupancy col) into voxel windows via
    one-hot matmuls on PE, then project 49->256 per window and write straight
    into the channel-major output layout (no transposes, no indirect DMA).
  - project occ 256->48 first (channel-major layout is already matmul-ready),
    PE-transpose the small projected volume into row-major [vox,48(+pad)] in
    DRAM, then dma_gather 512B x-pair rows per trilinear corner and do the
    weighted corner sum on DVE.

Sharding: 8 cores = 2 (b,t) x 4 y-quarters. One SPMD program; all per-core
differences are data (host-built index/weight tables from the point geometry).
"""

import sys

if "/opt/trn_rl_repo" not in sys.path:
    sys.path.insert(0, "/opt/trn_rl_repo")

import numpy as np

# ---------------- problem constants (hardcoded from spec) ----------------
GRID_MIN = np.array([-40.0, -40.0, -2.0], dtype=np.float32)
VOXEL = np.float32(0.8)
NZ, NY, NX = 10, 100, 100
NVOX = NZ * NY * NX
B, T, V, H, W_, Cg, Co = 1, 2, 6, 32, 88, 48, 256
BT, NPTS = B * T, V * H * W_  # 2, 16896

KQ = 4             # y-quarter cores per (b,t)
YQ = 25            # y rows owned per quarter (scatter)
YROWS = YQ + 1     # occP slice rows per z (gather needs y0+1)
ROWLEN = YROWS * NX          # 2600 contiguous elems per (ch,z) in occ_in
SLICE = NZ * ROWLEN          # 26000 rows in occP
SCROW = YQ * NX              # 2500 scatter-owned elems per z
PCH = [(c, min(512, ROWLEN - c)) for c in range(0, ROWLEN, 512)]   # proj chunks
SWIN = [(c, min(512, SCROW - c)) for c in range(0, SCROW, 512)]    # scatter windows
OCCP_COLS = 64     # 48 data + 16 pad so a (x,x0+1) pair row is 512B
F32 = np.float32


# ---------------- host-side geometry prep ----------------

def _voxel_ijk(pts):
    # identical formula to reference._scatter_mean, fp32 throughout
    q = (pts - GRID_MIN[None, :]) / VOXEL
    ijk = np.floor(q).astype(np.int32)
    return ijk


def _trilinear_geom(pts):
    # identical formula to reference._trilinear_sample, fp32 throughout
    dims = np.array([NX * 0.8, NY * 0.8, NZ * 0.8], dtype=np.float32)
    sizes = np.array([NX, NY, NZ], dtype=np.float32)
    g = (pts - GRID_MIN[None, :]) / np.maximum(dims, np.float32(1e-6)) * np.float32(2.0) - np.float32(1.0)
    pix = ((g + np.float32(1.0)) * sizes[None, :] - np.float32(1.0)) * np.float32(0.5)
    p0 = np.floor(pix)
    frac = (pix - p0).astype(np.float32)
    return p0.astype(np.int32), frac


def host_prep(centers, gs_features):
    """Build per-core input tensors + the (input-derived) static program meta."""
    pts = centers.reshape(BT, NPTS, 3)
    feats = gs_features.reshape(BT, NPTS, Cg)

    # ---- scatter geometry ----
    sc_buckets = {}   # (bt,k) -> dict (z,w) -> list of point idx
    sc_scale = np.zeros((BT, NPTS), dtype=F32)
    sc_zwu = np.zeros((BT, NPTS, 3), dtype=np.int32)   # z, w, urel per point
    sc_core = -np.ones((BT, NPTS), dtype=np.int32)
    for bt in range(BT):
        ijk = _voxel_ijk(pts[bt])
        ix, iy, iz = ijk[:, 0], ijk[:, 1], ijk[:, 2]
        valid = (ix >= 0) & (ix < NX) & (iy >= 0) & (iy < NY) & (iz >= 0) & (iz < NZ)
        lin = (np.clip(iz, 0, NZ - 1) * (NY * NX)
               + np.clip(iy, 0, NY - 1) * NX
               + np.clip(ix, 0, NX - 1))
        cnt = np.bincount(lin[valid], minlength=NVOX).astype(F32)
        sc_scale[bt, valid] = np.float32(1.0) / cnt[lin[valid]]
        k = iy // YQ
        uloc = (iy - k * YQ) * NX + ix
        w = np.minimum(uloc // 512, len(SWIN) - 1)
        sc_zwu[bt, :, 0] = iz
        sc_zwu[bt, :, 1] = w
        sc_zwu[bt, :, 2] = uloc - w * 512
        sc_core[bt, valid] = k[valid]
        for kk in range(KQ):
            d = {}
            sel = np.nonzero(valid & (k == kk))[0]
            for n in sel:
                d.setdefault((iz[n], w[n]), []).append(n)
            sc_buckets[(bt, kk)] = d

    # uniform rounds grid
    rgrid = np.zeros((NZ, len(SWIN)), dtype=np.int64)
    for z in range(NZ):
        for w in range(len(SWIN)):
            m = max(len(sc_buckets[(bt, kk)].get((z, w), []))
                    for bt in range(BT) for kk in range(KQ))
            rgrid[z, w] = max(1, -(-m // 128))
    # chunk meta in emission order: (z, w, w0, W, r, R)
    sc_chunks = []
    for z in range(NZ):
        for w, (w0, Ww) in enumerate(SWIN):
            R = int(rgrid[z, w])
            for r in range(R):
                sc_chunks.append((z, w, w0, Ww, r, R))
    NCHUNK_SC = len(sc_chunks)

    # ---- gather geometry ----
    g_bucket_pts = {}
    gp0 = np.zeros((BT, NPTS, 3), dtype=np.int32)
    gfr = np.zeros((BT, NPTS, 3), dtype=F32)
    for bt in range(BT):
        p0, frac = _trilinear_geom(pts[bt])
        gp0[bt], gfr[bt] = p0, frac
        k = np.clip(p0[:, 1] // YQ, 0, KQ - 1)
        for kk in range(KQ):
            g_bucket_pts[(bt, kk)] = np.nonzero(k == kk)[0]
    NT_G = max(-(-len(g_bucket_pts[(bt, kk)]) // 128)
               for bt in range(BT) for kk in range(KQ))
    NB_G = -(-NT_G // 4)

    meta = dict(sc_chunks=sc_chunks, NCHUNK_SC=NCHUNK_SC, NB_G=NB_G)

    # ---- per-core tensors ----
    in_maps = []
    perms = []
    for core in range(8):
        bt, kk = core // KQ, core % KQ

        # fsu: [NCHUNK_SC, 128, 52] = feats(48) | one(1) | scale(1) | urel(1) | pad(1)
        fsu = np.zeros((NCHUNK_SC, 128, 52), dtype=F32)
        bk = sc_buckets[(bt, kk)]
        for c, (z, w, w0, Ww, r, R) in enumerate(sc_chunks):
            lst = bk.get((z, w), [])
            part = lst[r * 128:(r + 1) * 128]
            if part:
                idxs = np.asarray(part, dtype=np.int64)
                npart = len(idxs)
                fsu[c, :npart, :Cg] = feats[bt, idxs]
                fsu[c, :npart, Cg] = 1.0
                fsu[c, :npart, Cg + 1] = sc_scale[bt, idxs]
                fsu[c, :npart, Cg + 2] = sc_zwu[bt, idxs, 2].astype(F32)

        # gather tables
        pidx = g_bucket_pts[(bt, kk)]
        perms.append(pidx)
        npt = len(pidx)
        gidx = np.zeros((NB_G, 128, 128), dtype=np.int16)
        gw = np.zeros((NB_G, 128, 32), dtype=F32)
        p0 = gp0[bt][pidx]
        fr = gfr[bt][pidx]
        x0, y0, z0 = p0[:, 0], p0[:, 1], p0[:, 2]
        fx, fy, fz = fr[:, 0], fr[:, 1], fr[:, 2]
        xp = np.clip(x0, 0, NX - 2)
        for cp, (dz, dy) in enumerate(((0, 0), (0, 1), (1, 0), (1, 1))):
            zc = z0 + dz
            yc = y0 + dy
            okzy = ((zc >= 0) & (zc < NZ) & (yc >= 0) & (yc < NY)).astype(F32)
            wz = np.where(dz == 1, fz, np.float32(1.0) - fz).astype(F32)
            wy = np.where(dy == 1, fy, np.float32(1.0) - fy).astype(F32)
            idx = (np.clip(zc, 0, NZ - 1) * ROWLEN
                   + np.clip(yc - kk * YQ, 0, YROWS - 1) * NX + xp).astype(np.int64)
            for s in range(2):
                xs = xp + s
                wx = np.where(xs == x0, np.float32(1.0) - fx,
                              np.where(xs == x0 + 1, fx, np.float32(0.0))).astype(F32)
                wfull = (wx * wy * wz * okzy).astype(F32)
                for n in range(npt):
                    tg = n // 128          # global tile
                    b = tg // 4            # batch
                    t = tg % 4             # tile in batch
                    p = n % 128
                    i = (t * 4 + cp) * 128 + p     # slot in batch
                    gw[b, p, t * 8 + cp * 2 + s] = wfull[n]
                    if s == 0:
                        gidx[b, (i % 16) + 16 * 0, i // 16] = idx[n]
            # replicate wrapped idx rows across the 8 gpsimd core groups
        for rr in range(1, 8):
            gidx[:, 16 * rr:16 * rr + 16, :] = gidx[:, 0:16, :]

        # occ_in filled by caller (needs occ_volume)
        in_maps.append(dict(fsu=fsu, gidx=gidx, gw=gw))

    return in_maps, perms, meta


# ---------------- device kernel builder ----------------

def build_kernel(meta):
    import concourse.bass as bass
    import concourse.mybir as mybir
    import concourse.tile as tile
    from concourse import bacc
    from concourse.masks import make_identity

    f32, f32r = mybir.dt.float32, mybir.dt.float32r
    i16, i32 = mybir.dt.int16, mybir.dt.int32
    AOP = mybir.AluOpType
    sc_chunks, NB_G = meta["sc_chunks"], meta["NB_G"]
    NCHUNK_SC = meta["NCHUNK_SC"]

    nc = bacc.Bacc(None, target_bir_lowering=False, num_swdge_queues=4)

    occ_in = nc.dram_tensor("occ_in", [Co, NZ, YROWS, NX], f32, kind="ExternalInput")
    fsu_t = nc.dram_tensor("fsu", [NCHUNK_SC, 128, 52], f32, kind="ExternalInput")
    gidx_t = nc.dram_tensor("gidx", [NB_G, 128, 128], i16, kind="ExternalInput")
    gw_t = nc.dram_tensor("gw", [NB_G, 128, 32], f32, kind="ExternalInput")
    wp_t = nc.dram_tensor("wp", [Cg + 1, Co], f32, kind="ExternalInput")
    wo_t = nc.dram_tensor("wo", [2, 128, Cg], f32, kind="ExternalInput")
    brep_t = nc.dram_tensor("brep", [128, 4, Cg], f32, kind="ExternalInput")

    out_sc = nc.dram_tensor("out_sc", [2, 128, NZ, SCROW], f32, kind="ExternalOutput")
    out_g = nc.dram_tensor("out_g", [NB_G * 512, Cg], f32, kind="ExternalOutput")
    occp = nc.dram_tensor("occp", [SLICE + 1, OCCP_COLS], f32, kind="Internal")

    with tile.TileContext(nc) as tc:
        with (
            tc.tile_pool(name="const", bufs=1) as cpool,
            tc.tile_pool(name="proj", bufs=3) as ppool,
            tc.tile_pool(name="scat", bufs=3) as spool,
            tc.tile_pool(name="gath", bufs=2) as gpool,
            tc.tile_pool(name="ps_proj", bufs=2, space="PSUM") as ps_proj,
            tc.tile_pool(name="ps_scat", bufs=2, space="PSUM") as ps_scat,
        ):
            # ---- constants ----
            ident = cpool.tile([128, 128], f32)
            make_identity(nc, ident[:])
            iota_i = cpool.tile([128, 512], i32)
            nc.gpsimd.iota(iota_i[:], pattern=[[1, 512]], base=0, channel_multiplier=0)
            iota_f = cpool.tile([128, 512], f32)
            nc.vector.tensor_copy(iota_f[:], iota_i[:])
            wp = cpool.tile([Cg + 1, Co], f32)
            nc.sync.dma_start(wp[:], wp_t[:])
            wo = cpool.tile([128, 2, Cg], f32)
            nc.sync.dma_start(wo[:], wo_t[:].rearrange("h p c -> p h c"))
            brep = cpool.tile([128, 4, Cg], f32)
            nc.sync.dma_start(brep[:], brep_t[:])
            zrow = cpool.tile([1, OCCP_COLS], f32)
            nc.gpsimd.memset(zrow[:], 0.0)
            nc.sync.dma_start(
                bass.AP(occp[:].tensor, SLICE * OCCP_COLS, [[1, OCCP_COLS]]), zrow[:])

            occ_view = occ_in[:].rearrange("(h c) z y x -> c h z (y x)", h=2)

            # ---- occ -> occP projection (256 -> 48, then transpose to rows) ----
            for z in range(NZ):
                for c0, Wc in PCH:
                    oc = ppool.tile([128, 2, Wc], f32, tag="oc")
                    nc.sync.dma_start(oc[:], occ_view[:, :, z, c0:c0 + Wc])
                    pj = ps_proj.tile([Cg, Wc], f32, tag="pj")
                    for h in range(2):
                        nc.tensor.matmul(
                            pj[:],
                            lhsT=wo[:, h, :].bitcast(f32r),
                            rhs=oc[:, h, :].bitcast(f32r),
                            start=(h == 0), stop=(h == 1),
                        )
                    s1 = ppool.tile([Cg, Wc], f32, tag="s1")
                    nc.scalar.copy(s1[:], pj[:])
                    nb = -(-Wc // 128)
                    pT = ps_proj.tile([128, nb, Cg], f32, tag="pT")
                    for t in range(nb):
                        wseg = min(128, Wc - t * 128)
                        nc.tensor.transpose(
                            pT[:wseg, t, :],
                            s1[:, t * 128:t * 128 + wseg],
                            ident[:Cg, :Cg],
                        )
                    sT = ppool.tile([128, nb, OCCP_COLS], f32, tag="sT")
                    nc.gpsimd.memset(sT[:, :, Cg:], 0.0)
                    r0 = z * ROWLEN + c0
                    if Wc % 128 == 0:
                        nc.vector.tensor_copy(sT[:, :, :Cg], pT[:])
                        dst = bass.AP(occp[:].tensor, r0 * OCCP_COLS,
                                      [[OCCP_COLS, 128], [128 * OCCP_COLS, nb], [1, OCCP_COLS]])
                        nc.sync.dma_start(dst, sT[:])
                    else:
                        wseg = Wc - (nb - 1) * 128
                        nc.vector.tensor_copy(sT[:wseg, nb - 1, :Cg], pT[:wseg, nb - 1, :])
                        if nb > 1:
                            nc.vector.tensor_copy(sT[:, :nb - 1, :Cg], pT[:, :nb - 1, :])
                            dst = bass.AP(occp[:].tensor, r0 * OCCP_COLS,
                                          [[OCCP_COLS, 128], [128 * OCCP_COLS, nb - 1], [1, OCCP_COLS]])
                            nc.sync.dma_start(dst, sT[:, :nb - 1, :])
                        dstr = bass.AP(occp[:].tensor, (r0 + (nb - 1) * 128) * OCCP_COLS,
                                       [[OCCP_COLS, wseg], [1, OCCP_COLS]])
                        nc.sync.dma_start(dstr, sT[:wseg, nb - 1, :])

            # ---- scatter: one-hot matmul accumulate + project per window ----
            widx = 0
            for ci, (z, w, w0, Ww, r, R) in enumerate(sc_chunks):
                ft = spool.tile([128, 52], f32, tag="ft")
                nc.sync.dma_start(ft[:], fsu_t[ci])
                fs = spool.tile([128, Cg + 1], f32, tag="fs")
                nc.vector.tensor_scalar(
                    out=fs[:], in0=ft[:, :Cg + 1],
                    scalar1=ft[:, Cg + 1:Cg + 2], scalar2=None, op0=AOP.mult)
                oh = spool.tile([128, Ww], f32, tag="oh")
                nc.vector.tensor_scalar(
                    out=oh[:], in0=iota_f[:, :Ww],
                    scalar1=ft[:, Cg + 2:Cg + 3], scalar2=None, op0=AOP.is_equal)
                if r == 0:
                    psw = ps_scat.tile([Cg + 1, Ww], f32, tag="psw")
                    meta.setdefault("_psw", {})[ci - r] = psw
                psw = meta["_psw"][ci - r]
                nc.tensor.matmul(
                    psw[:], lhsT=fs[:].bitcast(f32r), rhs=oh[:].bitcast(f32r),
                    start=(r == 0), stop=(r == R - 1))
                if r == R - 1:
                    ws = spool.tile([Cg + 1, Ww], f32, tag="ws")
                    nc.scalar.copy(ws[:], psw[:])
                    for h in range(2):
                        p2 = ps_scat.tile([128, Ww], f32, tag="p2")
                        nc.tensor.matmul(
                            p2[:], lhsT=wp[:, h * 128:(h + 1) * 128].bitcast(f32r),
                            rhs=ws[:].bitcast(f32r), start=True, stop=True)
                        o2 = spool.tile([128, Ww], f32, tag="o2")
                        if h == 0:
                            nc.scalar.copy(o2[:], p2[:])
                        else:
                            nc.vector.tensor_copy(o2[:], p2[:])
                        nc.sync.dma_start(out_sc[h, :, z, w0:w0 + Ww], o2[:])
                    widx += 1

            # ---- gather: dma_gather corner pairs + DVE weighted sum ----
            gsrc = bass.AP(occp[:].tensor, 0, [[OCCP_COLS, SLICE], [1, 128]])
            for b in range(NB_G):
                it = gpool.tile([128, 128], i16, tag="it")
                nc.sync.dma_start(it[:], gidx_t[b])
                wt = gpool.tile([128, 32], f32, tag="wt")
                nc.sync.dma_start(wt[:], gw_t[b])
                g = gpool.tile([128, 16, 128], f32, tag="g")
                nc.gpsimd.dma_gather(
                    g[:], gsrc, it[:], num_idxs=2048, num_idxs_reg=2048,
                    elem_size=128, elem_step=OCCP_COLS, queue_num=b % 4)
                gv = g[:].rearrange("p (t cp) e -> p t cp e", cp=4)
                wv = wt[:].rearrange("p (t q) -> p t q", q=8)
                acc = gpool.tile([128, 4, Cg], f32, tag="acc")
                tmp = gpool.tile([128, 4, Cg], f32, tag="tmp")
                first = True
                for cp in range(4):
                    for s in range(2):
                        gsl = gv[:, :, cp, s * 64:s * 64 + Cg]
                        wsl = wv[:, :, cp * 2 + s:cp * 2 + s + 1].to_broadcast([128, 4, Cg])
                        if first:
                            nc.vector.tensor_tensor(out=acc[:], in0=gsl, in1=wsl, op=AOP.mult)
                            first = False
                        else:
                            nc.vector.tensor_tensor(out=tmp[:], in0=gsl, in1=wsl, op=AOP.mult)
                            nc.vector.tensor_tensor(out=acc[:], in0=acc[:], in1=tmp[:], op=AOP.add)
                nc.vector.tensor_tensor(out=acc[:], in0=acc[:], in1=brep[:], op=AOP.add)
                dstg = out_g[:].rearrange("(b t p) c -> b p t c", t=4, p=128)[b]
                nc.sync.dma_start(dstg, acc[:])

    meta.pop("_psw", None)
    nc.compile()
    return nc


# ---------------- top-level entry ----------------

def kernel(**inputs):
    centers = np.asarray(inputs["centers"], dtype=np.float32)
    gs_features = np.asarray(inputs["gs_features"], dtype=np.float32)
    occ_volume = np.asarray(inputs["occ_volume"], dtype=np.float32)
    W_g2o = np.asarray(inputs["W_g2o"], dtype=np.float32)
    b_g2o = np.asarray(inputs["b_g2o"], dtype=np.float32)
    W_o2g = np.asarray(inputs["W_o2g"], dtype=np.float32)
    b_o2g = np.asarray(inputs["b_o2g"], dtype=np.float32)

    in_maps, perms, meta = host_prep(centers, gs_features)

    wp = np.concatenate([W_g2o, b_g2o[None, :]], axis=0).astype(F32)       # [49,256]
    wo = W_o2g.reshape(2, 128, Cg).astype(F32)                             # [2,128,48]
    brep = np.broadcast_to(b_o2g[None, None, :], (128, 4, Cg)).astype(F32).copy()

    occ = occ_volume.reshape(BT, Co, NZ, NY, NX)
    for core in range(8):
        bt, kk = core // KQ, core % KQ
        sl = np.zeros((Co, NZ, YROWS, NX), dtype=F32)
        ylo = kk * YQ
        yhi = min(ylo + YROWS, NY)
        sl[:, :, :yhi - ylo, :] = occ[bt, :, :, ylo:yhi, :]
        in_maps[core].update(occ_in=sl, wp=wp, wo=wo, brep=brep)

    nc = build_kernel(meta)

    from concourse import bass_utils
    res = bass_utils.run_bass_kernel_spmd(nc, in_maps, core_ids=list(range(8)))

    gs_to_occ = np.zeros((B, T, Co, NZ, NY, NX), dtype=F32)
    occ_to_gs_flat = np.zeros((BT, NPTS, Cg), dtype=F32)
    for core in range(8):
        bt, kk = core // KQ, core % KQ
        osc = res.results[core]["out_sc"].reshape(Co, NZ, YQ, NX)
        gs_to_occ[0, bt, :, :, kk * YQ:(kk + 1) * YQ, :] = osc
        og = res.results[core]["out_g"]
        pidx = perms[core]
        occ_to_gs_flat[bt, pidx] = og[:len(pidx)]
    occ_to_gs = occ_to_gs_flat.reshape(B, T, V, H, W_, Cg)
    return gs_to_occ, occ_to_gs
